# revision 2
# baseline (speedup 1.0000x reference)
"""Single-head causal attention on 8 TRN2 NeuronCores.

Problem shapes (hardcoded): B=8, T=2048, C=1024, H=64, fp32 I/O.
    q = x @ Wq; k = x @ Wk; v = x @ Wv          (per batch element)
    wei = softmax(causal_mask(q @ k.T * C**-0.5))
    out = wei @ v
Sharding: pure data parallel - one batch element per core, no collectives.

Per-core algorithm (bf16 matmuls, fp32 PSUM accumulation):
  - host pre-transposes x -> xT [C, T] and packs [Wq|Wk]; per 512-wide
    T-slice: qkT = [Wq|Wk].T @ xT, vT = Wv.T @ xT.
  - S^T row-packed: kT2 holds Tk-block pairs in the partition halves,
    qT2hi duplicates q into the hi half; h0 reads q straight from qkT.
    The two halves of an S pair run CONCURRENTLY (row groups h0/h64).
    All half-shuffles are partition-shifted ENGINE copies (Pool for
    SBUF->SBUF, DVE to drain PSUM).
  - exp always one WIDE ACT per [128,1024] pair tile; columns outside
    the causal n0 window hold garbage that AV never reads.  P = exp(S/32)
    with no max-subtraction; diagonal blocks masked 0/1 on Pool, with the
    masks scheduled as standalone items right before the AV that needs
    them (keeps Pool head-of-line free for the qT2hi/kT2 shifts).
  - v1 = [v | 1] -> [num|den] share one accumulator.  v natural is
    recovered by row-packed identity matmuls (VTR) whose four outputs
    land in ONE ps_big tile (lo row-group -> bank 0 cols 0:128, hi ->
    bank 1 cols 512:640) so concurrent drains never share a bank.
  - EPILOGUE IS HOST-SIDE: the [65,512] av accumulator is copied f32 ->
    SBUF and DMA'd per-slice to a [65,T] output (2KB descriptors); the
    num/den divide and [H,T]->[T,H] transpose happen in numpy.  This
    removes all epilogue matmuls/reciprocals from the device and keeps
    num/den in fp32 end to end.
  - THE SCHEDULE IS A FLAT GLOBAL INTERLEAVE tuned so ScalarE exp
    (~20 x 1.1us) never starves: projections run as early as the input
    DMA allows (QK3 right after QK2), S pair tiles are emitted densely,
    AV/V/VTR/EPn fill the PE between them.  PSUM pools rotate
    deadlock-free: ps_big = S pairs + VTR tiles (2 bufs), ps_av =
    v_ps/av alternating, ps_mix = qk tiles only.
  - 14 dummy warmup matmuls release the HAM clock gate (PE starts at
    1.2 GHz, reaches 2.4 only after ~3.4us of sustained activity) while
    the input DMAs stream; v1's memset is split so the warmup operand
    (v1[:,0:4]) is ready ~250ns after the preamble barrier.
  - HW-DGE queues carry only inputs + stores (16 DMA instructions);
    xT streams in T-quarter x C-half chunks, the two halves of each
    quarter on the two queues concurrently.
"""

import numpy as np
import ml_dtypes

import concourse.bass as bass
import concourse.mybir as mybir
import concourse.tile as tile
from concourse import bacc
from concourse.bass_utils import run_bass_kernel_spmd

B, T, C, H = 8, 2048, 1024, 64
NCB = C // 128          # 8 C-blocks
NT = T // 128           # 16 Tk-blocks of 128
NJ = T // 512           # 4 Tq-slices of 512
SCALE = float(C) ** -0.5  # 1/32

BF16 = mybir.dt.bfloat16
F32 = mybir.dt.float32
npbf16 = ml_dtypes.bfloat16


class Ctx:
    pass


def build_attention(nc: bass.Bass, tc: tile.TileContext, ctx):
    g = Ctx()
    g.nc = nc
    xT_d = nc.dram_tensor("xT", [128, NCB, T], BF16,
                          kind="ExternalInput").ap()
    wqk_d = nc.dram_tensor("wqk", [128, NCB, 128], BF16,
                           kind="ExternalInput").ap()
    wv_d = nc.dram_tensor("wv", [128, NCB, H], BF16,
                          kind="ExternalInput").ap()
    ident_d = nc.dram_tensor("idents", [128, 192], BF16,
                             kind="ExternalInput").ap()
    g.o65_d = nc.dram_tensor("o65", [65, T], F32, kind="ExternalOutput").ap()

    consts = ctx.enter_context(tc.tile_pool(name="consts", bufs=1))
    persist = ctx.enter_context(tc.tile_pool(name="persist", bufs=1))
    g.pts = ctx.enter_context(tc.tile_pool(name="pts", bufs=6))
    g.outts = ctx.enter_context(tc.tile_pool(name="outts", bufs=2))
    g.ps_big = ctx.enter_context(tc.tile_pool(name="ps_big", bufs=2,
                                              space="PSUM"))
    g.ps_av = ctx.enter_context(tc.tile_pool(name="ps_av", bufs=2,
                                             space="PSUM"))
    g.ps_mix = ctx.enter_context(tc.tile_pool(name="ps_mix", bufs=2,
                                              space="PSUM"))

    g.v1 = persist.tile([128, NT, H + 1], BF16, tag="v1")  # [v | 1]
    # split memset: warmup's operand region first (first DVE op after the
    # preamble barrier) so dummy matmuls start ~7.5us, then the rest.
    nc.vector.memset(g.v1[:, 0:4, :], 1.0)
    nc.vector.memset(g.v1[:, 4:NT, :], 1.0)

    # ---- input DMAs: minimal count on the two HW DGE queues, in
    # consumption order; both halves of each T-quarter stream concurrently.
    g.wqk_sb = consts.tile([128, NCB, 128], BF16, tag="wqk")
    g.xT_sb = persist.tile([128, NCB, T], BF16, tag="xT")
    g.wv_sb = consts.tile([128, NCB, H], BF16, tag="wv")
    ident_sb = consts.tile([128, 192], BF16, tag="idents")
    nc.sync.dma_start(out=g.wqk_sb[:, 0:4, :], in_=wqk_d[:, 0:4, :])
    nc.scalar.dma_start(out=g.wqk_sb[:, 4:8, :], in_=wqk_d[:, 4:8, :])
    nc.sync.dma_start(out=g.xT_sb[:, 0:4, 0:512], in_=xT_d[:, 0:4, 0:512])
    nc.scalar.dma_start(out=g.xT_sb[:, 4:8, 0:512], in_=xT_d[:, 4:8, 0:512])
    nc.scalar.dma_start(out=g.wv_sb, in_=wv_d)
    nc.scalar.dma_start(out=ident_sb, in_=ident_d)
    for qa in range(1, 4):
        qs = slice(qa * 512, (qa + 1) * 512)
        nc.sync.dma_start(out=g.xT_sb[:, 0:4, qs], in_=xT_d[:, 0:4, qs])
        nc.scalar.dma_start(out=g.xT_sb[:, 4:8, qs], in_=xT_d[:, 4:8, qs])

    g.i64_sb = ident_sb[:, 0:64]
    g.causal_sb = ident_sb[:, 64:192]

    g.qkT = persist.tile([128, T], BF16, tag="qkT")      # [q; k]
    g.qT2hi = persist.tile([128, T], BF16, tag="qT2hi")  # q in rows 64:128
    g.kT2 = persist.tile([128, T // 2], BF16, tag="kT2")
    g.vT = persist.tile([64, T], BF16, tag="vT")
    g.vT2 = persist.tile([128, T // 2], BF16, tag="vT2")

    g.s_pend = [[] for _ in range(NJ)]
    g.avs = [None] * NJ

    # ---- flat global schedule ------------------------------------------
    QK, V, S, M, VTR, AV, EP = (emit_qk, emit_v, emit_s, emit_mask,
                                emit_vtr, emit_avu, emit_epn)
    QK(g, 0, warmup=14)
    V(g, 0)
    S(g, 0, 0)
    S(g, 0, 1)
    QK(g, 1)
    M(g, 0, 0)
    M(g, 0, 1)
    VTR(g, 0)
    AV(g, 0, 0)
    AV(g, 0, 1)
    EP(g, 0, nc.sync)
    V(g, 1)
    S(g, 1, 0)
    S(g, 1, 1)
    QK(g, 2)
    AV(g, 1, 0)
    S(g, 1, 2)
    AV(g, 1, 1)
    VTR(g, 1)
    S(g, 1, 3)
    QK(g, 3)
    M(g, 1, 2)
    AV(g, 1, 2)
    S(g, 2, 0)
    M(g, 1, 3)
    AV(g, 1, 3)
    EP(g, 1, nc.scalar)
    V(g, 2)
    S(g, 2, 1)
    AV(g, 2, 0)
    S(g, 2, 2)
    AV(g, 2, 1)
    VTR(g, 2)
    S(g, 2, 3)
    AV(g, 2, 2)
    S(g, 2, 4)
    V(g, 3)
    AV(g, 2, 3)
    S(g, 3, 0)
    M(g, 2, 4)
    AV(g, 2, 4)
    S(g, 2, 5)
    M(g, 2, 5)
    AV(g, 2, 5)
    EP(g, 2, nc.sync)
    VTR(g, 3)
    S(g, 3, 1)
    AV(g, 3, 0)
    S(g, 3, 2)
    AV(g, 3, 1)
    S(g, 3, 3)
    AV(g, 3, 2)
    S(g, 3, 4)
    AV(g, 3, 3)
    S(g, 3, 5)
    AV(g, 3, 4)
    S(g, 3, 6)
    M(g, 3, 6)
    AV(g, 3, 5)
    S(g, 3, 7)
    M(g, 3, 7)
    AV(g, 3, 6)
    AV(g, 3, 7)
    EP(g, 3, nc.scalar)


def emit_qk(g, j, warmup=0):
    nc = g.nc
    jsl = slice(j * 512, (j + 1) * 512)
    qk_ps = g.ps_mix.tile([128, 512], F32, tag="mix", name=f"qk_ps{j}")
    for w in range(warmup):  # HAM warmup; first real matmul resets PSUM
        nc.tensor.matmul(qk_ps[0:65, 0:260], lhsT=g.v1[:, 0, :],
                         rhs=g.v1[:, 0:4, :], start=True, stop=True,
                         skip_group_check=True)
    for c in range(NCB):
        nc.tensor.matmul(qk_ps, lhsT=g.wqk_sb[:, c, :],
                         rhs=g.xT_sb[:, c, jsl],
                         start=(c == 0), stop=(c == NCB - 1))
    nc.vector.tensor_copy(g.qkT[:, jsl], qk_ps)
    # odd k-blocks (4j+1, 4j+3) straight from PSUM into kT2 hi half
    for b in (1, 3):
        c0 = (2 * j + b // 2) * 128
        nc.vector.tensor_copy(g.kT2[64:128, c0:c0 + 128],
                              qk_ps[64:128, b * 128:(b + 1) * 128])
    # partition-shifted SBUF copies on Pool: q dup, even k-blocks
    nc.gpsimd.tensor_copy(g.qT2hi[64:128, jsl], g.qkT[0:64, jsl])
    for b in (0, 2):
        c0 = (2 * j + b // 2) * 128
        nc.gpsimd.tensor_copy(
            g.kT2[0:64, c0:c0 + 128],
            g.qkT[64:128, j * 512 + b * 128:j * 512 + (b + 1) * 128])


def emit_v(g, j):
    nc = g.nc
    jsl = slice(j * 512, (j + 1) * 512)
    v_ps = g.ps_av.tile([128, 512], F32, tag="av", name=f"v_ps{j}")
    for c in range(NCB):
        nc.tensor.matmul(v_ps[0:64, :], lhsT=g.wv_sb[:, c, :],
                         rhs=g.xT_sb[:, c, jsl],
                         start=(c == 0), stop=(c == NCB - 1))
    # cast + odd-block shift here (no PE work) so v_ps frees early and the
    # transpose slot later has its inputs ready
    nc.vector.tensor_copy(g.vT[:, jsl], v_ps[0:64, :])
    for bb in range(2):  # odd Tk blocks -> vT2 hi half (Pool shift)
        tb = 4 * j + 2 * bb + 1
        c0 = (2 * j + bb) * 128
        nc.gpsimd.tensor_copy(g.vT2[64:128, c0:c0 + 128],
                              g.vT[:, tb * 128:(tb + 1) * 128])


def emit_s(g, j, m):
    """Row-packed S^T pair tile (k-blocks 2m, 2m+1): one wide exp."""
    nc = g.nc
    sp2 = g.ps_big.tile([128, 1024], F32, tag="big", name=f"sp{j}_{m}")
    pt2 = g.pts.tile([128, 1024], BF16, tag="pt", name=f"pt{j}_{m}")
    n0s = []
    for half_idx, i in ((0, 2 * m), (1, 2 * m + 1)):
        g_ = i - 4 * j
        n0 = max(0, g_) * 128
        p0 = half_idx * 64
        o = half_idx * 512
        rhs = (g.qkT if half_idx == 0 else g.qT2hi)
        nc.tensor.matmul(
            sp2[:, o + n0:o + 512],
            lhsT=g.kT2[p0:p0 + 64, m * 128:(m + 1) * 128],
            rhs=rhs[p0:p0 + 64, j * 512 + n0:(j + 1) * 512],
            start=True, stop=True)
        n0s.append(n0)
    # wide exp over both banks; cols below n0 are garbage nobody reads
    nc.scalar.activation(pt2, sp2, mybir.ActivationFunctionType.Exp,
                         scale=SCALE)
    g.s_pend[j].append((pt2, n0s, 2 * m))


def emit_mask(g, j, m):
    """0/1 triangular mask on the diagonal blocks of pair tile (j, m).
    Standalone schedule item so Pool's head-of-line stays free."""
    nc = g.nc
    e = None
    for idx, (pt2, n0s, i0) in enumerate(g.s_pend[j]):
        if i0 == 2 * m:
            e = idx
            break
    pt2, n0s, i0 = g.s_pend[j][e]
    for half_idx, i in ((0, 2 * m), (1, 2 * m + 1)):
        if i - 4 * j >= 0:  # mask upper triangle of the diagonal block
            o = half_idx * 512 + n0s[half_idx]
            nc.gpsimd.tensor_mul(
                pt2[:, o:o + 128], pt2[:, o:o + 128], g.causal_sb)


def emit_vtr(g, j):
    """transpose v back into v1 = [v|1] via row-packed identity matmuls.
    All four outputs land in one ps_big tile: lo row-group -> bank 0
    (cols 0:128), hi row-group -> bank 1 (cols 512:640), so the two
    concurrent matmuls of a pair never drain into the same bank."""
    nc = g.nc
    vp = g.ps_big.tile([128, 1024], F32, tag="big", name=f"vp{j}")
    for u, mt in enumerate((2 * j, 2 * j + 1)):
        tA, tB = 2 * mt, 2 * mt + 1
        nc.tensor.matmul(vp[:, u * 64:(u + 1) * 64],
                         lhsT=g.vT[:, tA * 128:(tA + 1) * 128],
                         rhs=g.i64_sb[0:64, :], start=True, stop=True)
        nc.tensor.matmul(vp[:, 512 + u * 64:512 + (u + 1) * 64],
                         lhsT=g.vT2[64:128, mt * 128:(mt + 1) * 128],
                         rhs=g.i64_sb[64:128, :], start=True, stop=True)
    for u in range(2):
        nc.vector.tensor_copy(g.v1[:, 4 * j + 2 * u, 0:H],
                              vp[:, u * 64:(u + 1) * 64])
        nc.vector.tensor_copy(g.v1[:, 4 * j + 2 * u + 1, 0:H],
                              vp[:, 512 + u * 64:512 + (u + 1) * 64])


def emit_avu(g, j, e):
    """AV accumulation for the e-th EMITTED pair tile of slice j."""
    nc = g.nc
    if e == 0:
        g.avs[j] = g.ps_av.tile([65, 512], F32, tag="av", name=f"av{j}")
    av = g.avs[j]
    pt2, n0s, i0 = g.s_pend[j][e]
    last = 2 * j + 1
    for d in range(2):
        o, n0 = d * 512, n0s[d]
        nc.tensor.matmul(av[:, n0:512], lhsT=g.v1[:, i0 + d, :],
                         rhs=pt2[:, o + n0:o + 512],
                         start=(e == 0 and d == 0), stop=(e == last and d == 1))


def emit_epn(g, j, dq):
    """Per-slice epilogue: drain [num|den] f32 to SBUF and store.  The
    divide + transpose happen on the host."""
    nc = g.nc
    jsl = slice(j * 512, (j + 1) * 512)
    osb = g.outts.tile([65, 512], F32, tag="osb", name=f"osb{j}")
    nc.vector.tensor_copy(osb, g.avs[j])
    dq.dma_start(out=g.o65_d[:, jsl], in_=osb)


_CACHED = {}


def _get_nc(n=B):
    key = ("nc", n)
    if key not in _CACHED:
        from contextlib import ExitStack
        nc = bacc.Bacc("TRN2", target_bir_lowering=False, debug=False,
                       num_devices=n)
        with tile.TileContext(nc) as tc:
            with ExitStack() as ctx:
                build_attention(nc, tc, ctx)
        nc.compile()
        _CACHED[key] = nc
    return _CACHED[key]


def _quant_inputs(inputs, Wq, Wk, Wv):
    """Host-side prep: xT in [128, 8, T] bf16 layout, packed [Wq|Wk]."""
    inputs = np.asarray(inputs, dtype=np.float32)

    def wlayout(w, m):  # [C, m] -> [128, 8, m]
        return np.ascontiguousarray(
            np.asarray(w).astype(npbf16).reshape(8, 128, m).transpose(
                1, 0, 2))

    wqk = wlayout(np.concatenate([np.asarray(Wq), np.asarray(Wk)], axis=1),
                  128)
    wv = wlayout(Wv, H)

    idents = np.zeros((128, 192), dtype=npbf16)
    idents[0:64, 0:64] = np.eye(64, dtype=npbf16)
    idents[64:128, 0:64] = np.eye(64, dtype=npbf16)
    idents[:, 64:192] = np.triu(np.ones((128, 128), dtype=npbf16))

    in_maps = []
    for b in range(inputs.shape[0]):
        xT = np.ascontiguousarray(
            inputs[b].T.astype(npbf16).reshape(8, 128, T).transpose(1, 0, 2))
        in_maps.append({"xT": xT, "wqk": wqk, "wv": wv, "idents": idents})
    return in_maps


def _gather_out(res, n=B):
    """[65,T] per core -> [n,T,H]: host-side num/den divide + transpose."""
    outs = []
    for b in range(n):
        o65 = np.asarray(res.results[b]["o65"], dtype=np.float32)
        outs.append((o65[0:64] / o65[64:65]).T)
    return np.ascontiguousarray(np.stack(outs, axis=0).astype(np.float32))


def _spot_check(out, x, Wq, Wk, Wv):
    """Cheap host-side corruption detector: recompute one output row per
    128-row block per batch in fp32 numpy and compare.  The bf16 kernel
    sits at ~1e-2 per-row error; transient device corruption (observed
    ~2/50 executions after long run streaks: one all-NaN, one 2.5e-2
    global) blows individual rows far past 0.1."""
    wq = np.asarray(Wq, np.float32)
    wk = np.asarray(Wk, np.float32)
    wv = np.asarray(Wv, np.float32)
    scale = float(C) ** -0.5
    rows = np.arange(64, T, 128)
    for b in range(B):
        K = x[b] @ wk
        V = x[b] @ wv
        for t in rows:
            q = x[b, t] @ wq
            s = (K[: t + 1] @ q) * scale
            p = np.exp(s - s.max())
            p /= p.sum()
            ref = p @ V[: t + 1]
            err = np.linalg.norm(out[b, t] - ref) / np.linalg.norm(ref)
            if not np.isfinite(err) or err > 0.1:
                return False
    return True


def kernel(inputs, Wq, Wk, Wv):
    x = np.asarray(inputs, dtype=np.float32)
    in_maps = _quant_inputs(x, Wq, Wk, Wv)
    nc = _get_nc()
    for _attempt in range(3):
        res = run_bass_kernel_spmd(nc, in_maps, core_ids=list(range(B)))
        out = _gather_out(res)
        if _spot_check(out, x, Wq, Wk, Wv):
            break
    return out


# revision 7
# speedup vs baseline: 1.2695x; 1.2695x over previous
"""Single-head causal attention on 8 TRN2 NeuronCores.

Problem shapes (hardcoded): B=8, T=2048, C=1024, H=64, fp32 I/O.
    q = x @ Wq; k = x @ Wk; v = x @ Wv          (per batch element)
    wei = softmax(causal_mask(q @ k.T * C**-0.5))
    out = wei @ v
Sharding: pure data parallel - one batch element per core, no collectives.

Per-core algorithm (bf16 matmuls, fp32 PSUM accumulation):
  - host pre-transposes x -> xT [C, T] and packs [Wq|Wk]; per 512-wide
    T-slice: qkT = [Wq|Wk].T @ xT, vT = Wv.T @ xT.
  - S^T row-packed: kT2 holds Tk-block pairs in the partition halves,
    qT2hi duplicates q into the hi half; h0 reads q straight from qkT.
    The two halves of an S pair run CONCURRENTLY (row groups h0/h64).
    All half-shuffles are partition-shifted ENGINE copies (Pool for
    SBUF->SBUF, DVE to drain PSUM).
  - exp always one WIDE ACT per [128,1024] pair tile; columns outside
    the causal n0 window hold garbage that AV never reads.  P = exp(S/32)
    with no max-subtraction; diagonal blocks masked 0/1 on Pool, with the
    masks scheduled as standalone items right before the AV that needs
    them (keeps Pool head-of-line free for the qT2hi/kT2 shifts).
  - v1 = [v | 1] -> [num|den] share one accumulator.  v natural is
    recovered by row-packed identity matmuls (VTR) whose four outputs
    land in ONE ps_big tile (lo row-group -> bank 0 cols 0:128, hi ->
    bank 1 cols 512:640) so concurrent drains never share a bank.
  - EPILOGUE IS HOST-SIDE: the [65,512] av accumulator is copied f32 ->
    SBUF and DMA'd per-slice to a [65,T] output (2KB descriptors); the
    num/den divide and [H,T]->[T,H] transpose happen in numpy.  This
    removes all epilogue matmuls/reciprocals from the device and keeps
    num/den in fp32 end to end.
  - THE SCHEDULE IS A FLAT GLOBAL INTERLEAVE tuned so ScalarE exp
    (~20 x 1.1us) never starves: projections run as early as the input
    DMA allows (QK3 right after QK2), S pair tiles are emitted densely,
    AV/V/VTR/EPn fill the PE between them.  PSUM pools rotate
    deadlock-free: ps_big = S pairs + VTR tiles (2 bufs), ps_av =
    v_ps/av alternating, ps_mix = qk tiles only.
  - 14 dummy warmup matmuls release the HAM clock gate (PE starts at
    1.2 GHz, reaches 2.4 only after ~3.4us of sustained activity) while
    the input DMAs stream; v1's memset is split so the warmup operand
    (v1[:,0:4]) is ready ~250ns after the preamble barrier.
  - HW-DGE queues carry only inputs + stores (16 DMA instructions);
    xT streams in T-quarter x C-half chunks, the two halves of each
    quarter on the two queues concurrently.
"""

import numpy as np
import ml_dtypes

import concourse.bass as bass
import concourse.mybir as mybir
import concourse.tile as tile
from concourse import bacc
from concourse.bass_utils import run_bass_kernel_spmd

B, T, C, H = 8, 2048, 1024, 64
NCB = C // 128          # 8 C-blocks
NT = T // 128           # 16 Tk-blocks of 128
NJ = T // 512           # 4 Tq-slices of 512
SCALE = float(C) ** -0.5  # 1/32

BF16 = mybir.dt.bfloat16
F32 = mybir.dt.float32
npbf16 = ml_dtypes.bfloat16


class Ctx:
    pass


def build_attention(nc: bass.Bass, tc: tile.TileContext, ctx):
    g = Ctx()
    g.nc = nc
    xT_d = nc.dram_tensor("xT", [128, NCB, T], BF16,
                          kind="ExternalInput").ap()
    wqk_d = nc.dram_tensor("wqk", [128, NCB, 128], BF16,
                           kind="ExternalInput").ap()
    wv_d = nc.dram_tensor("wv", [128, NCB, H], BF16,
                          kind="ExternalInput").ap()
    ident_d = nc.dram_tensor("idents", [128, 192], BF16,
                             kind="ExternalInput").ap()
    g.o65_d = nc.dram_tensor("o65", [65, T], F32, kind="ExternalOutput").ap()

    consts = ctx.enter_context(tc.tile_pool(name="consts", bufs=1))
    persist = ctx.enter_context(tc.tile_pool(name="persist", bufs=1))
    g.pts = ctx.enter_context(tc.tile_pool(name="pts", bufs=6))
    g.outts = ctx.enter_context(tc.tile_pool(name="outts", bufs=2))
    g.ps_big = ctx.enter_context(tc.tile_pool(name="ps_big", bufs=2,
                                              space="PSUM"))
    g.ps_av = ctx.enter_context(tc.tile_pool(name="ps_av", bufs=2,
                                             space="PSUM"))
    g.ps_mix = ctx.enter_context(tc.tile_pool(name="ps_mix", bufs=2,
                                              space="PSUM"))

    g.v1 = persist.tile([128, NT, H + 1], BF16, tag="v1")  # [v | 1]
    # split memset: warmup's operand region first (first DVE op after the
    # preamble barrier) so dummy matmuls start ~7.5us, then the rest.
    nc.vector.memset(g.v1[:, 0:4, :], 1.0)
    nc.vector.memset(g.v1[:, 4:NT, :], 1.0)

    # ---- input DMAs: minimal count on the two HW DGE queues, in
    # consumption order; both halves of each T-quarter stream concurrently.
    g.wqk_sb = consts.tile([128, NCB, 128], BF16, tag="wqk")
    g.xT_sb = persist.tile([128, NCB, T], BF16, tag="xT")
    g.wv_sb = consts.tile([128, NCB, H], BF16, tag="wv")
    ident_sb = consts.tile([128, 192], BF16, tag="idents")
    # sync: x0lo first (QK0 c0-3 gate), then small consts, then x-lo quarters.
    # scalar: wqk-lo first (tiny), then all x-hi quarters back to back.
    nc.scalar.dma_start(out=g.wqk_sb[:, 0:4, :], in_=wqk_d[:, 0:4, :])
    nc.sync.dma_start(out=g.xT_sb[:, 0:4, 0:512], in_=xT_d[:, 0:4, 0:512])
    nc.scalar.dma_start(out=g.xT_sb[:, 4:8, 0:512], in_=xT_d[:, 4:8, 0:512])
    nc.sync.dma_start(out=ident_sb, in_=ident_d)
    nc.sync.dma_start(out=g.wv_sb, in_=wv_d)
    nc.sync.dma_start(out=g.wqk_sb[:, 4:8, :], in_=wqk_d[:, 4:8, :])
    for qa in range(1, 4):
        qs = slice(qa * 512, (qa + 1) * 512)
        nc.sync.dma_start(out=g.xT_sb[:, 0:4, qs], in_=xT_d[:, 0:4, qs])
        nc.scalar.dma_start(out=g.xT_sb[:, 4:8, qs], in_=xT_d[:, 4:8, qs])

    g.i64_sb = ident_sb[:, 0:64]
    g.causal_sb = ident_sb[:, 64:192]

    g.qkT = persist.tile([64, T], BF16, tag="qkT")       # q rows only
    g.qT2hi = persist.tile([128, T], BF16, tag="qT2hi")  # q in rows 64:128
    g.kT2 = persist.tile([128, T // 2], BF16, tag="kT2")
    g.vT = persist.tile([64, T], BF16, tag="vT")
    g.vT2 = persist.tile([128, T // 2], BF16, tag="vT2")

    g.s_pend = [[] for _ in range(NJ)]
    g.avs = [None] * NJ

    # ---- flat global schedule ------------------------------------------
    QK, V, S, M, VTR, AV, EP = (emit_qk, emit_v, emit_s, emit_mask,
                                emit_vtr, emit_avu, emit_epn)
    QK(g, 0, warmup=16)
    V(g, 0)
    S(g, 0, 0)
    S(g, 0, 1)
    QK(g, 1)
    M(g, 0, 0)
    M(g, 0, 1)
    VTR(g, 0)
    S(g, 1, 0)
    S(g, 1, 1)
    V(g, 1)
    AV(g, 0, 0)
    S(g, 1, 2)
    AV(g, 0, 1)
    QK(g, 2)
    VTR(g, 1)
    S(g, 1, 3)
    EP(g, 0, nc.sync)
    AV(g, 1, 0)
    AV(g, 1, 1)
    S(g, 2, 0)
    S(g, 2, 1)
    QK(g, 3)
    M(g, 1, 2)
    AV(g, 1, 2)
    M(g, 1, 3)
    AV(g, 1, 3)
    EP(g, 1, nc.scalar)
    V(g, 2)
    S(g, 2, 2)
    AV(g, 2, 0)
    S(g, 2, 3)
    AV(g, 2, 1)
    VTR(g, 2)
    S(g, 2, 4)
    AV(g, 2, 2)
    S(g, 3, 0)
    V(g, 3)
    M(g, 2, 4)
    AV(g, 2, 3)
    S(g, 2, 5)
    M(g, 2, 5)
    AV(g, 2, 4)
    AV(g, 2, 5)
    EP(g, 2, nc.sync)
    VTR(g, 3)
    S(g, 3, 1)
    AV(g, 3, 0)
    S(g, 3, 2)
    AV(g, 3, 1)
    S(g, 3, 3)
    AV(g, 3, 2)
    S(g, 3, 4)
    AV(g, 3, 3)
    S(g, 3, 5)
    AV(g, 3, 4)
    S(g, 3, 6)
    M(g, 3, 6)
    AV(g, 3, 5)
    S(g, 3, 7)
    M(g, 3, 7)
    AV(g, 3, 6)
    AV(g, 3, 7)
    EP(g, 3, nc.scalar)


def emit_qk(g, j, warmup=0):
    """[q;k] projection.  ALL drains run on DVE straight from PSUM:
    q -> qkT rows 0:64 and (shifted) qT2hi rows 64:128; k even blocks
    (shifted) -> kT2 lo, k odd blocks -> kT2 hi, merged as strided
    2-free-dim copies.  No Pool work at all."""
    nc = g.nc
    jsl = slice(j * 512, (j + 1) * 512)
    qk_ps = g.ps_mix.tile([128, 512], F32, tag="mix", name=f"qk_ps{j}")
    for w in range(warmup):  # HAM warmup; first real matmul resets PSUM
        nc.tensor.matmul(qk_ps[0:65, 0:260], lhsT=g.v1[:, 0, :],
                         rhs=g.v1[:, 0:4, :], start=True, stop=True,
                         skip_group_check=True)
    order = range(NCB) if j == 0 else list(range(4, 8)) + list(range(4))
    for ci, c in enumerate(order):
        nc.tensor.matmul(qk_ps, lhsT=g.wqk_sb[:, c, :],
                         rhs=g.xT_sb[:, c, jsl],
                         start=(ci == 0), stop=(ci == NCB - 1))
    c0 = j * 256
    nc.vector.tensor_copy(g.qkT[:, jsl], qk_ps[0:64, :])
    # even k-blocks (4j, 4j+2): PSUM hi -> kT2 lo half (partition shift)
    nc.vector.tensor_copy(
        g.kT2[0:64, c0:c0 + 256].rearrange("p (b n) -> p b n", b=2),
        qk_ps[64:128, :].rearrange("p (b h n) -> p b h n", b=2, h=2)[:, :, 0])
    nc.vector.tensor_copy(g.qT2hi[64:128, jsl], qk_ps[0:64, :])
    # odd k-blocks (4j+1, 4j+3): PSUM hi -> kT2 hi half (aligned)
    nc.vector.tensor_copy(
        g.kT2[64:128, c0:c0 + 256].rearrange("p (b n) -> p b n", b=2),
        qk_ps[64:128, :].rearrange("p (b h n) -> p b h n", b=2, h=2)[:, :, 1])


def emit_v(g, j):
    nc = g.nc
    jsl = slice(j * 512, (j + 1) * 512)
    v_ps = g.ps_av.tile([128, 512], F32, tag="av", name=f"v_ps{j}")
    order = range(NCB) if j == 0 else list(range(4, 8)) + list(range(4))
    for ci, c in enumerate(order):
        nc.tensor.matmul(v_ps[0:64, :], lhsT=g.wv_sb[:, c, :],
                         rhs=g.xT_sb[:, c, jsl],
                         start=(ci == 0), stop=(ci == NCB - 1))
    # drains on DVE: natural cast + odd-block partition shift, from PSUM
    nc.vector.tensor_copy(g.vT[:, jsl], v_ps[0:64, :])
    nc.vector.tensor_copy(
        g.vT2[64:128, j * 256:j * 256 + 256].rearrange(
            "p (b n) -> p b n", b=2),
        v_ps[0:64, :].rearrange("p (b h n) -> p b h n", b=2, h=2)[:, :, 1])


def emit_s(g, j, m):
    """Row-packed S^T pair tile (k-blocks 2m, 2m+1): one wide exp."""
    nc = g.nc
    sp2 = g.ps_big.tile([128, 1024], F32, tag="big", name=f"sp{j}_{m}")
    pt2 = g.pts.tile([128, 1024], BF16, tag="pt", name=f"pt{j}_{m}")
    n0s = []
    for half_idx, i in ((0, 2 * m), (1, 2 * m + 1)):
        g_ = i - 4 * j
        n0 = max(0, g_) * 128
        p0 = half_idx * 64
        o = half_idx * 512
        rhs = (g.qkT if half_idx == 0 else g.qT2hi)
        nc.tensor.matmul(
            sp2[:, o + n0:o + 512],
            lhsT=g.kT2[p0:p0 + 64, m * 128:(m + 1) * 128],
            rhs=rhs[p0:p0 + 64, j * 512 + n0:(j + 1) * 512],
            start=True, stop=True)
        n0s.append(n0)
    # wide exp over both banks; cols below n0 are garbage nobody reads
    nc.scalar.activation(pt2, sp2, mybir.ActivationFunctionType.Exp,
                         scale=SCALE)
    g.s_pend[j].append((pt2, n0s, 2 * m))


def emit_mask(g, j, m):
    """0/1 triangular mask on the diagonal blocks of pair tile (j, m).
    Standalone schedule item so Pool's head-of-line stays free."""
    nc = g.nc
    e = None
    for idx, (pt2, n0s, i0) in enumerate(g.s_pend[j]):
        if i0 == 2 * m:
            e = idx
            break
    pt2, n0s, i0 = g.s_pend[j][e]
    for half_idx, i in ((0, 2 * m), (1, 2 * m + 1)):
        if i - 4 * j >= 0:  # mask upper triangle of the diagonal block
            o = half_idx * 512 + n0s[half_idx]
            nc.gpsimd.tensor_mul(
                pt2[:, o:o + 128], pt2[:, o:o + 128], g.causal_sb)


def emit_vtr(g, j):
    """transpose v back into v1 = [v|1] via row-packed identity matmuls.
    All four outputs land in one ps_big tile: lo row-group -> bank 0
    (cols 0:128), hi row-group -> bank 1 (cols 512:640), so the two
    concurrent matmuls of a pair never drain into the same bank."""
    nc = g.nc
    vp = g.ps_big.tile([128, 1024], F32, tag="big", name=f"vp{j}")
    for u, mt in enumerate((2 * j, 2 * j + 1)):
        tA, tB = 2 * mt, 2 * mt + 1
        nc.tensor.matmul(vp[:, u * 64:(u + 1) * 64],
                         lhsT=g.vT[:, tA * 128:(tA + 1) * 128],
                         rhs=g.i64_sb[0:64, :], start=True, stop=True)
        nc.tensor.matmul(vp[:, 512 + u * 64:512 + (u + 1) * 64],
                         lhsT=g.vT2[64:128, mt * 128:(mt + 1) * 128],
                         rhs=g.i64_sb[64:128, :], start=True, stop=True)
    dst = g.v1[:, 4 * j:4 * j + 4, 0:H].rearrange(
        "p (b o) h -> p b o h", b=2, o=2)
    nc.vector.tensor_copy(
        dst[:, :, 0], vp[:, 0:128].rearrange("p (b h) -> p b h", b=2))
    nc.vector.tensor_copy(
        dst[:, :, 1], vp[:, 512:640].rearrange("p (b h) -> p b h", b=2))


def emit_avu(g, j, e):
    """AV accumulation for the e-th EMITTED pair tile of slice j."""
    nc = g.nc
    if e == 0:
        g.avs[j] = g.ps_av.tile([65, 512], F32, tag="av", name=f"av{j}")
    av = g.avs[j]
    pt2, n0s, i0 = g.s_pend[j][e]
    last = 2 * j + 1
    for d in range(2):
        o, n0 = d * 512, n0s[d]
        nc.tensor.matmul(av[:, n0:512], lhsT=g.v1[:, i0 + d, :],
                         rhs=pt2[:, o + n0:o + 512],
                         start=(e == 0 and d == 0), stop=(e == last and d == 1))


def emit_epn(g, j, dq):
    """Per-slice epilogue: drain [num|den] f32 to SBUF and store.  The
    divide + transpose happen on the host."""
    nc = g.nc
    jsl = slice(j * 512, (j + 1) * 512)
    osb = g.outts.tile([65, 512], F32, tag="osb", name=f"osb{j}")
    nc.vector.tensor_copy(osb, g.avs[j])
    dq.dma_start(out=g.o65_d[:, jsl], in_=osb)


_CACHED = {}


def _get_nc(n=B):
    key = ("nc", n)
    if key not in _CACHED:
        from contextlib import ExitStack
        nc = bacc.Bacc("TRN2", target_bir_lowering=False, debug=False,
                       num_devices=n)
        with tile.TileContext(nc) as tc:
            with ExitStack() as ctx:
                build_attention(nc, tc, ctx)
        nc.compile()
        _CACHED[key] = nc
    return _CACHED[key]


def _quant_inputs(inputs, Wq, Wk, Wv):
    """Host-side prep: xT in [128, 8, T] bf16 layout, packed [Wq|Wk]."""
    inputs = np.asarray(inputs, dtype=np.float32)

    def wlayout(w, m):  # [C, m] -> [128, 8, m]
        return np.ascontiguousarray(
            np.asarray(w).astype(npbf16).reshape(8, 128, m).transpose(
                1, 0, 2))

    wqk = wlayout(np.concatenate([np.asarray(Wq), np.asarray(Wk)], axis=1),
                  128)
    wv = wlayout(Wv, H)

    idents = np.zeros((128, 192), dtype=npbf16)
    idents[0:64, 0:64] = np.eye(64, dtype=npbf16)
    idents[64:128, 0:64] = np.eye(64, dtype=npbf16)
    idents[:, 64:192] = np.triu(np.ones((128, 128), dtype=npbf16))

    in_maps = []
    for b in range(inputs.shape[0]):
        xT = np.ascontiguousarray(
            inputs[b].T.astype(npbf16).reshape(8, 128, T).transpose(1, 0, 2))
        in_maps.append({"xT": xT, "wqk": wqk, "wv": wv, "idents": idents})
    return in_maps


def _gather_out(res, n=B):
    """[65,T] per core -> [n,T,H]: host-side num/den divide + transpose."""
    outs = []
    for b in range(n):
        o65 = np.asarray(res.results[b]["o65"], dtype=np.float32)
        outs.append((o65[0:64] / o65[64:65]).T)
    return np.ascontiguousarray(np.stack(outs, axis=0).astype(np.float32))


def _spot_check(out, x, Wq, Wk, Wv):
    """Cheap host-side corruption detector: recompute one output row per
    128-row block per batch in fp32 numpy and compare.  The bf16 kernel
    sits at ~1e-2 per-row error; transient device corruption (observed
    ~2/50 executions after long run streaks: one all-NaN, one 2.5e-2
    global) blows individual rows far past 0.1."""
    wq = np.asarray(Wq, np.float32)
    wk = np.asarray(Wk, np.float32)
    wv = np.asarray(Wv, np.float32)
    scale = float(C) ** -0.5
    rows = np.arange(64, T, 128)
    for b in range(B):
        K = x[b] @ wk
        V = x[b] @ wv
        for t in rows:
            q = x[b, t] @ wq
            s = (K[: t + 1] @ q) * scale
            p = np.exp(s - s.max())
            p /= p.sum()
            ref = p @ V[: t + 1]
            err = np.linalg.norm(out[b, t] - ref) / np.linalg.norm(ref)
            if not np.isfinite(err) or err > 0.1:
                return False
    return True


def kernel(inputs, Wq, Wk, Wv):
    x = np.asarray(inputs, dtype=np.float32)
    in_maps = _quant_inputs(x, Wq, Wk, Wv)
    nc = _get_nc()
    for _attempt in range(3):
        res = run_bass_kernel_spmd(nc, in_maps, core_ids=list(range(B)))
        out = _gather_out(res)
        if _spot_check(out, x, Wq, Wk, Wv):
            break
    return out


# revision 8
# speedup vs baseline: 1.3399x; 1.0555x over previous
"""Single-head causal attention on 8 TRN2 NeuronCores.

Problem shapes (hardcoded): B=8, T=2048, C=1024, H=64, fp32 I/O.
    q = x @ Wq; k = x @ Wk; v = x @ Wv          (per batch element)
    wei = softmax(causal_mask(q @ k.T * C**-0.5))
    out = wei @ v
Sharding: pure data parallel - one batch element per core, no collectives.

Per-core algorithm (bf16 matmuls, fp32 PSUM accumulation):
  - host pre-transposes x -> xT [C, T] and packs [Wq|Wk]; per 512-wide
    T-slice: qkT = [Wq|Wk].T @ xT, vT = Wv.T @ xT.
  - S^T row-packed: kT2 holds Tk-block pairs in the partition halves,
    qT2hi duplicates q into the hi half; h0 reads q straight from qkT.
    The two halves of an S pair run CONCURRENTLY (row groups h0/h64).
    All half-shuffles are partition-shifted ENGINE copies (Pool for
    SBUF->SBUF, DVE to drain PSUM).
  - exp always one WIDE ACT per [128,1024] pair tile; columns outside
    the causal n0 window hold garbage that AV never reads.  P = exp(S/32)
    with no max-subtraction; diagonal blocks masked 0/1 on Pool, with the
    masks scheduled as standalone items right before the AV that needs
    them (keeps Pool head-of-line free for the qT2hi/kT2 shifts).
  - v1 = [v | 1] -> [num|den] share one accumulator.  v natural is
    recovered by row-packed identity matmuls (VTR) whose four outputs
    land in ONE ps_big tile (lo row-group -> bank 0 cols 0:128, hi ->
    bank 1 cols 512:640) so concurrent drains never share a bank.
  - EPILOGUE IS HOST-SIDE: the [65,512] av accumulator is copied f32 ->
    SBUF and DMA'd per-slice to a [65,T] output (2KB descriptors); the
    num/den divide and [H,T]->[T,H] transpose happen in numpy.  This
    removes all epilogue matmuls/reciprocals from the device and keeps
    num/den in fp32 end to end.
  - THE SCHEDULE IS A FLAT GLOBAL INTERLEAVE tuned so ScalarE exp
    (~20 x 1.1us) never starves: projections run as early as the input
    DMA allows (QK3 right after QK2), S pair tiles are emitted densely,
    AV/V/VTR/EPn fill the PE between them.  PSUM pools rotate
    deadlock-free: ps_big = S pairs + VTR tiles (2 bufs), ps_av =
    v_ps/av alternating, ps_mix = qk tiles only.
  - 14 dummy warmup matmuls release the HAM clock gate (PE starts at
    1.2 GHz, reaches 2.4 only after ~3.4us of sustained activity) while
    the input DMAs stream; v1's memset is split so the warmup operand
    (v1[:,0:4]) is ready ~250ns after the preamble barrier.
  - HW-DGE queues carry only inputs + stores (16 DMA instructions);
    xT streams in T-quarter x C-half chunks, the two halves of each
    quarter on the two queues concurrently.
"""

import numpy as np
import ml_dtypes

import concourse.bass as bass
import concourse.mybir as mybir
import concourse.tile as tile
from concourse import bacc
from concourse.bass_utils import run_bass_kernel_spmd

B, T, C, H = 8, 2048, 1024, 64
NCB = C // 128          # 8 C-blocks
NT = T // 128           # 16 Tk-blocks of 128
NJ = T // 512           # 4 Tq-slices of 512
SCALE = float(C) ** -0.5  # 1/32

BF16 = mybir.dt.bfloat16
F32 = mybir.dt.float32
npbf16 = ml_dtypes.bfloat16


class Ctx:
    pass


def build_attention(nc: bass.Bass, tc: tile.TileContext, ctx):
    g = Ctx()
    g.nc = nc
    xT_d = nc.dram_tensor("xT", [128, NCB, T], BF16,
                          kind="ExternalInput").ap()
    wqk_d = nc.dram_tensor("wqk", [128, NCB, 128], BF16,
                           kind="ExternalInput").ap()
    wv_d = nc.dram_tensor("wv", [128, NCB, H], BF16,
                          kind="ExternalInput").ap()
    ident_d = nc.dram_tensor("idents", [128, 192], BF16,
                             kind="ExternalInput").ap()
    g.o65_d = nc.dram_tensor("o65", [65, T], F32, kind="ExternalOutput").ap()

    consts = ctx.enter_context(tc.tile_pool(name="consts", bufs=1))
    persist = ctx.enter_context(tc.tile_pool(name="persist", bufs=1))
    g.pts = ctx.enter_context(tc.tile_pool(name="pts", bufs=6))
    g.outts = ctx.enter_context(tc.tile_pool(name="outts", bufs=2))
    g.ps_big = ctx.enter_context(tc.tile_pool(name="ps_big", bufs=2,
                                              space="PSUM"))
    g.ps_av = ctx.enter_context(tc.tile_pool(name="ps_av", bufs=2,
                                             space="PSUM"))
    g.ps_mix = ctx.enter_context(tc.tile_pool(name="ps_mix", bufs=2,
                                              space="PSUM"))

    g.v1 = persist.tile([128, NT, H + 1], BF16, tag="v1")  # [v | 1]
    # split memset: warmup's operand region first (first DVE op after the
    # preamble barrier) so dummy matmuls start ~7.5us, then the rest.
    nc.vector.memset(g.v1[:, 0:4, :], 1.0)
    nc.vector.memset(g.v1[:, 4:NT, :], 1.0)

    # ---- input DMAs: minimal count on the two HW DGE queues, in
    # consumption order; both halves of each T-quarter stream concurrently.
    g.wqk_sb = consts.tile([128, NCB, 128], BF16, tag="wqk")
    g.xT_sb = persist.tile([128, NCB, T], BF16, tag="xT")
    g.wv_sb = consts.tile([128, NCB, H], BF16, tag="wv")
    ident_sb = consts.tile([128, 192], BF16, tag="idents")
    # scalar: both wqk halves (tiny) then all x-hi quarters back to back.
    # sync: x0lo first (QK0 c0-3 gate), wv+idents, then x-lo quarters.
    nc.scalar.dma_start(out=g.wqk_sb[:, 0:4, :], in_=wqk_d[:, 0:4, :])
    nc.sync.dma_start(out=g.xT_sb[:, 0:4, 0:512], in_=xT_d[:, 0:4, 0:512])
    nc.scalar.dma_start(out=g.wqk_sb[:, 4:8, :], in_=wqk_d[:, 4:8, :])
    nc.scalar.dma_start(out=g.xT_sb[:, 4:8, 0:512], in_=xT_d[:, 4:8, 0:512])
    nc.sync.dma_start(out=g.wv_sb, in_=wv_d)
    nc.sync.dma_start(out=ident_sb, in_=ident_d)
    for qa in range(1, 4):
        qs = slice(qa * 512, (qa + 1) * 512)
        nc.sync.dma_start(out=g.xT_sb[:, 0:4, qs], in_=xT_d[:, 0:4, qs])
        nc.scalar.dma_start(out=g.xT_sb[:, 4:8, qs], in_=xT_d[:, 4:8, qs])

    g.i64_sb = ident_sb[:, 0:64]
    g.causal_sb = ident_sb[:, 64:192]

    g.qkT = persist.tile([64, T], BF16, tag="qkT")       # q rows only
    g.qT2hi = persist.tile([128, T], BF16, tag="qT2hi")  # q in rows 64:128
    g.kT2 = persist.tile([128, T // 2], BF16, tag="kT2")
    g.vT = persist.tile([64, T], BF16, tag="vT")
    g.vT2 = persist.tile([128, T // 2], BF16, tag="vT2")

    g.s_pend = [[] for _ in range(NJ)]
    g.avs = [None] * NJ

    # ---- flat global schedule ------------------------------------------
    QK, V, S, M, VTR, AV, EP = (emit_qk, emit_v, emit_s, emit_mask,
                                emit_vtr, emit_avu, emit_epn)
    QK(g, 0, warmup=16)
    V(g, 0)
    S(g, 0, 0)
    S(g, 0, 1)
    QK(g, 1)
    M(g, 0, 0)
    M(g, 0, 1)
    VTR(g, 0)
    S(g, 1, 0)
    S(g, 1, 1)
    V(g, 1)
    AV(g, 0, 0)
    S(g, 1, 2)
    AV(g, 0, 1)
    QK(g, 2)
    VTR(g, 1)
    S(g, 1, 3)
    EP(g, 0, nc.sync)
    AV(g, 1, 0)
    AV(g, 1, 1)
    S(g, 2, 0)
    S(g, 2, 1)
    QK(g, 3)
    M(g, 1, 2)
    AV(g, 1, 2)
    M(g, 1, 3)
    AV(g, 1, 3)
    EP(g, 1, nc.scalar)
    V(g, 2)
    S(g, 2, 2)
    AV(g, 2, 0)
    S(g, 2, 3)
    AV(g, 2, 1)
    VTR(g, 2)
    S(g, 2, 4)
    AV(g, 2, 2)
    S(g, 3, 0)
    V(g, 3)
    M(g, 2, 4)
    AV(g, 2, 3)
    S(g, 2, 5)
    M(g, 2, 5)
    AV(g, 2, 4)
    AV(g, 2, 5)
    EP(g, 2, nc.sync)
    VTR(g, 3)
    S(g, 3, 1)
    AV(g, 3, 0)
    S(g, 3, 2)
    AV(g, 3, 1)
    S(g, 3, 3)
    AV(g, 3, 2)
    S(g, 3, 4)
    AV(g, 3, 3)
    S(g, 3, 5)
    AV(g, 3, 4)
    S(g, 3, 6)
    M(g, 3, 6)
    AV(g, 3, 5)
    S(g, 3, 7)
    M(g, 3, 7)
    AV(g, 3, 6)
    AV(g, 3, 7)
    EP(g, 3, nc.scalar)


def emit_qk(g, j, warmup=0):
    """[q;k] projection.  ALL drains run on DVE straight from PSUM:
    q -> qkT rows 0:64 and (shifted) qT2hi rows 64:128; k even blocks
    (shifted) -> kT2 lo, k odd blocks -> kT2 hi, merged as strided
    2-free-dim copies.  No Pool work at all."""
    nc = g.nc
    jsl = slice(j * 512, (j + 1) * 512)
    qk_ps = g.ps_mix.tile([128, 512], F32, tag="mix", name=f"qk_ps{j}")
    for w in range(warmup):  # HAM warmup; first real matmul resets PSUM
        nc.tensor.matmul(qk_ps[0:65, 0:260], lhsT=g.v1[:, 0, :],
                         rhs=g.v1[:, 0:4, :], start=True, stop=True,
                         skip_group_check=True)
    order = range(NCB) if j == 0 else list(range(4, 8)) + list(range(4))
    for ci, c in enumerate(order):
        nc.tensor.matmul(qk_ps, lhsT=g.wqk_sb[:, c, :],
                         rhs=g.xT_sb[:, c, jsl],
                         start=(ci == 0), stop=(ci == NCB - 1))
    c0 = j * 256
    nc.vector.tensor_copy(g.qkT[:, jsl], qk_ps[0:64, :])
    # even k-blocks (4j, 4j+2): PSUM hi -> kT2 lo half (partition shift)
    nc.vector.tensor_copy(
        g.kT2[0:64, c0:c0 + 256].rearrange("p (b n) -> p b n", b=2),
        qk_ps[64:128, :].rearrange("p (b h n) -> p b h n", b=2, h=2)[:, :, 0])
    nc.vector.tensor_copy(g.qT2hi[64:128, jsl], qk_ps[0:64, :])
    # odd k-blocks (4j+1, 4j+3): PSUM hi -> kT2 hi half (aligned)
    nc.vector.tensor_copy(
        g.kT2[64:128, c0:c0 + 256].rearrange("p (b n) -> p b n", b=2),
        qk_ps[64:128, :].rearrange("p (b h n) -> p b h n", b=2, h=2)[:, :, 1])


def emit_v(g, j):
    nc = g.nc
    jsl = slice(j * 512, (j + 1) * 512)
    v_ps = g.ps_av.tile([128, 512], F32, tag="av", name=f"v_ps{j}")
    order = range(NCB) if j == 0 else list(range(4, 8)) + list(range(4))
    for ci, c in enumerate(order):
        nc.tensor.matmul(v_ps[0:64, :], lhsT=g.wv_sb[:, c, :],
                         rhs=g.xT_sb[:, c, jsl],
                         start=(ci == 0), stop=(ci == NCB - 1))
    # drains on DVE: natural cast + odd-block partition shift, from PSUM
    nc.vector.tensor_copy(g.vT[:, jsl], v_ps[0:64, :])
    nc.vector.tensor_copy(
        g.vT2[64:128, j * 256:j * 256 + 256].rearrange(
            "p (b n) -> p b n", b=2),
        v_ps[0:64, :].rearrange("p (b h n) -> p b h n", b=2, h=2)[:, :, 1])


def emit_s(g, j, m):
    """Row-packed S^T pair tile (k-blocks 2m, 2m+1): one wide exp."""
    nc = g.nc
    sp2 = g.ps_big.tile([128, 1024], F32, tag="big", name=f"sp{j}_{m}")
    pt2 = g.pts.tile([128, 1024], BF16, tag="pt", name=f"pt{j}_{m}")
    n0s = []
    for half_idx, i in ((0, 2 * m), (1, 2 * m + 1)):
        g_ = i - 4 * j
        n0 = max(0, g_) * 128
        p0 = half_idx * 64
        o = half_idx * 512
        rhs = (g.qkT if half_idx == 0 else g.qT2hi)
        nc.tensor.matmul(
            sp2[:, o + n0:o + 512],
            lhsT=g.kT2[p0:p0 + 64, m * 128:(m + 1) * 128],
            rhs=rhs[p0:p0 + 64, j * 512 + n0:(j + 1) * 512],
            start=True, stop=True)
        n0s.append(n0)
    # wide exp over both banks; cols below n0 are garbage nobody reads
    nc.scalar.activation(pt2, sp2, mybir.ActivationFunctionType.Exp,
                         scale=SCALE)
    g.s_pend[j].append((pt2, n0s, 2 * m))


def emit_mask(g, j, m):
    """0/1 triangular mask on the diagonal blocks of pair tile (j, m).
    Standalone schedule item so Pool's head-of-line stays free."""
    nc = g.nc
    e = None
    for idx, (pt2, n0s, i0) in enumerate(g.s_pend[j]):
        if i0 == 2 * m:
            e = idx
            break
    pt2, n0s, i0 = g.s_pend[j][e]
    for half_idx, i in ((0, 2 * m), (1, 2 * m + 1)):
        if i - 4 * j >= 0:  # mask upper triangle of the diagonal block
            o = half_idx * 512 + n0s[half_idx]
            nc.gpsimd.tensor_mul(
                pt2[:, o:o + 128], pt2[:, o:o + 128], g.causal_sb)


def emit_vtr(g, j):
    """transpose v back into v1 = [v|1] via row-packed identity matmuls.
    All four outputs land in one ps_big tile: lo row-group -> bank 0
    (cols 0:128), hi row-group -> bank 1 (cols 512:640), so the two
    concurrent matmuls of a pair never drain into the same bank."""
    nc = g.nc
    vp = g.ps_big.tile([128, 1024], F32, tag="big", name=f"vp{j}")
    for u, mt in enumerate((2 * j, 2 * j + 1)):
        tA, tB = 2 * mt, 2 * mt + 1
        nc.tensor.matmul(vp[:, u * 64:(u + 1) * 64],
                         lhsT=g.vT[:, tA * 128:(tA + 1) * 128],
                         rhs=g.i64_sb[0:64, :], start=True, stop=True)
        nc.tensor.matmul(vp[:, 512 + u * 64:512 + (u + 1) * 64],
                         lhsT=g.vT2[64:128, mt * 128:(mt + 1) * 128],
                         rhs=g.i64_sb[64:128, :], start=True, stop=True)
    dst = g.v1[:, 4 * j:4 * j + 4, 0:H].rearrange(
        "p (b o) h -> p b o h", b=2, o=2)
    nc.vector.tensor_copy(
        dst[:, :, 0], vp[:, 0:128].rearrange("p (b h) -> p b h", b=2))
    nc.vector.tensor_copy(
        dst[:, :, 1], vp[:, 512:640].rearrange("p (b h) -> p b h", b=2))


def emit_avu(g, j, e):
    """AV accumulation for the e-th EMITTED pair tile of slice j."""
    nc = g.nc
    if e == 0:
        g.avs[j] = g.ps_av.tile([65, 512], F32, tag="av", name=f"av{j}")
    av = g.avs[j]
    pt2, n0s, i0 = g.s_pend[j][e]
    last = 2 * j + 1
    for d in range(2):
        o, n0 = d * 512, n0s[d]
        nc.tensor.matmul(av[:, n0:512], lhsT=g.v1[:, i0 + d, :],
                         rhs=pt2[:, o + n0:o + 512],
                         start=(e == 0 and d == 0), stop=(e == last and d == 1))


def emit_epn(g, j, dq):
    """Per-slice epilogue: drain [num|den] f32 to SBUF and store.  The
    divide + transpose happen on the host."""
    nc = g.nc
    jsl = slice(j * 512, (j + 1) * 512)
    osb = g.outts.tile([65, 512], F32, tag="osb", name=f"osb{j}")
    nc.vector.tensor_copy(osb, g.avs[j])
    dq.dma_start(out=g.o65_d[:, jsl], in_=osb)


_CACHED = {}


def _get_nc(n=B):
    key = ("nc", n)
    if key not in _CACHED:
        from contextlib import ExitStack
        nc = bacc.Bacc("TRN2", target_bir_lowering=False, debug=False,
                       num_devices=n)
        with tile.TileContext(nc) as tc:
            with ExitStack() as ctx:
                build_attention(nc, tc, ctx)
        nc.compile()
        _CACHED[key] = nc
    return _CACHED[key]


def _quant_inputs(inputs, Wq, Wk, Wv):
    """Host-side prep: xT in [128, 8, T] bf16 layout, packed [Wq|Wk]."""
    inputs = np.asarray(inputs, dtype=np.float32)

    def wlayout(w, m):  # [C, m] -> [128, 8, m]
        return np.ascontiguousarray(
            np.asarray(w).astype(npbf16).reshape(8, 128, m).transpose(
                1, 0, 2))

    wqk = wlayout(np.concatenate([np.asarray(Wq), np.asarray(Wk)], axis=1),
                  128)
    wv = wlayout(Wv, H)

    idents = np.zeros((128, 192), dtype=npbf16)
    idents[0:64, 0:64] = np.eye(64, dtype=npbf16)
    idents[64:128, 0:64] = np.eye(64, dtype=npbf16)
    idents[:, 64:192] = np.triu(np.ones((128, 128), dtype=npbf16))

    in_maps = []
    for b in range(inputs.shape[0]):
        xT = np.ascontiguousarray(
            inputs[b].T.astype(npbf16).reshape(8, 128, T).transpose(1, 0, 2))
        in_maps.append({"xT": xT, "wqk": wqk, "wv": wv, "idents": idents})
    return in_maps


def _gather_out(res, n=B):
    """[65,T] per core -> [n,T,H]: host-side num/den divide + transpose."""
    outs = []
    for b in range(n):
        o65 = np.asarray(res.results[b]["o65"], dtype=np.float32)
        outs.append((o65[0:64] / o65[64:65]).T)
    return np.ascontiguousarray(np.stack(outs, axis=0).astype(np.float32))


def _spot_check(out, x, Wq, Wk, Wv):
    """Cheap host-side corruption detector: recompute one output row per
    128-row block per batch in fp32 numpy and compare.  The bf16 kernel
    sits at ~1e-2 per-row error; transient device corruption (observed
    ~2/50 executions after long run streaks: one all-NaN, one 2.5e-2
    global) blows individual rows far past 0.1."""
    wq = np.asarray(Wq, np.float32)
    wk = np.asarray(Wk, np.float32)
    wv = np.asarray(Wv, np.float32)
    scale = float(C) ** -0.5
    rows = np.arange(64, T, 128)
    for b in range(B):
        K = x[b] @ wk
        V = x[b] @ wv
        for t in rows:
            q = x[b, t] @ wq
            s = (K[: t + 1] @ q) * scale
            p = np.exp(s - s.max())
            p /= p.sum()
            ref = p @ V[: t + 1]
            err = np.linalg.norm(out[b, t] - ref) / np.linalg.norm(ref)
            if not np.isfinite(err) or err > 0.1:
                return False
    return True


def kernel(inputs, Wq, Wk, Wv):
    x = np.asarray(inputs, dtype=np.float32)
    in_maps = _quant_inputs(x, Wq, Wk, Wv)
    nc = _get_nc()
    for _attempt in range(3):
        res = run_bass_kernel_spmd(nc, in_maps, core_ids=list(range(B)))
        out = _gather_out(res)
        if _spot_check(out, x, Wq, Wk, Wv):
            break
    return out
